# revision 16
# baseline (speedup 1.0000x reference)
"""MoE top-1 routing kernel for Trainium2 (8 NeuronCores, expert-parallel).

Problem: x[65536,1024] fp32; gate = softmax(x @ Wg.T + bg); idx = argmax(gate);
out[n] = x[n] @ We[idx[n]].T + be[idx[n]].

The end-to-end wall time is dominated by the ~35-70 MB/s axon tunnel, so the
design minimizes host<->device bytes:

  Host (cheap: gating GEMM is 2 GFLOP):
    - fp32 routing: logits = x @ Wg.T + bg, idx = argmax (bit-exact fp32, so
      routing matches the reference; device bf16 gating would misroute).
    - per-token int8 quantization of x (per-row absmax scales), into
      preallocated buffers (fresh 256MB allocations cost ~1s/call).
    - counting-sort dispatch: core c owns experts 2c, 2c+1; tokens for each
      expert are packed into a static-capacity slot block (CAP_E per expert).
      Capacity overflow (never hit at these shapes) falls back to host numpy.
  Device (per core, all static, no collectives):
    - 72 token tiles of 128; tiles [0,36) use expert slot 0, [36,72) slot 1.
    - per tile: int8 load -> bf16 convert -> 8 PE transposes (k-major lhsT)
      -> 16 bf16 matmuls (2 psum halves, 8 k-chunks) -> scale by per-token
      input scale (ACT engine) -> +bias -> per-token abs-max -> uint8
      requantize (offset 128) -> store; row scales are a 2nd output.
  Transfers: x int8 (75 MB) + scales + We bf16 pair-sharded (4 MB/core) up;
  out uint8 (75 MB) + row scales down. Donated output buffers are created
  on-device (jnp.zeros jit), not shipped. Weight device buffers are cached
  across calls keyed on array equality.
"""
import os
import time
import threading
import numpy as np
import ml_dtypes

import jax
import jax.numpy as jnp
from jax.sharding import Mesh, PartitionSpec, NamedSharding

import concourse.bass as bass
import concourse.mybir as mybir
import concourse.tile as tile
from concourse import bacc
from concourse import bass2jax as _b2j
from concourse.masks import make_identity

P = 128
N_CORES = 8
N_TOK = 65536
D = 1024                      # d_in = d_out
E = 16
KC = D // P                   # 8 k-chunks
EPC = E // N_CORES            # 2 experts per core
CAP_E = 4224                  # token capacity per expert (33 tiles); overflow
                              # tokens (a few dozen at these shapes) are
                              # computed on host
CAP_C = EPC * CAP_E           # 9216 tokens per core
NTILE = CAP_C // P            # 72
NT_E = CAP_E // P             # 36
QBIAS = 128.0                 # uint8 quant offset (convert rounds to nearest)
QMAX = 126.5                  # max quantized magnitude

FP32 = mybir.dt.float32
FP16 = mybir.dt.float16
BF16 = mybir.dt.bfloat16
I8 = mybir.dt.int8
U8 = mybir.dt.uint8

_STATE: dict = {}


def build_nc():
    nc = bacc.Bacc("TRN2", target_bir_lowering=False, debug=False,
                   enable_asserts=False, num_devices=1)

    xq = nc.dram_tensor("xq", [CAP_C, D], I8, kind="ExternalInput")
    sxT = nc.dram_tensor("sxT", [P, NTILE], FP32, kind="ExternalInput")
    # wePT[s][p][c*D+d] = We[expert(s)][d, c*128+p]  (lhsT layout, host-prepped)
    wePT = nc.dram_tensor("wePT", [EPC, P, KC * D], BF16, kind="ExternalInput")
    beP = nc.dram_tensor("beP", [EPC, P, D], FP32, kind="ExternalInput")
    out = nc.dram_tensor("out", [CAP_C, D], U8, kind="ExternalOutput")
    soT = nc.dram_tensor("soT", [P, NTILE], FP32, kind="ExternalOutput")

    with tile.TileContext(nc) as tc:
        with tc.tile_pool(name="cst", bufs=1) as cst, \
             tc.tile_pool(name="xin", bufs=3) as xin, \
             tc.tile_pool(name="xbp", bufs=2) as xbp, \
             tc.tile_pool(name="gxp", bufs=2) as gxp, \
             tc.tile_pool(name="ofp", bufs=2) as ofp, \
             tc.tile_pool(name="yab", bufs=2) as yap, \
             tc.tile_pool(name="sc", bufs=3) as scp, \
             tc.tile_pool(name="op", bufs=3) as op, \
             tc.tile_pool(name="pt", bufs=4, space="PSUM") as pt, \
             tc.tile_pool(name="pm", bufs=2, space="PSUM") as pm:
            ident = cst.tile([P, P], BF16)
            make_identity(nc, ident[:])
            sx_sb = cst.tile([P, NTILE], FP32)
            nc.sync.dma_start(sx_sb[:], sxT[:])
            so_all = cst.tile([P, NTILE], FP32)
            w_sb = cst.tile([P, EPC, KC, D], BF16)
            for s in range(EPC):
                nc.sync.dma_start(
                    w_sb[:, s, :, :].rearrange("p c d -> p (c d)"), wePT[s])
            be_sb = cst.tile([P, EPC, D], FP32)
            for s in range(EPC):
                nc.sync.dma_start(be_sb[:, s, :], beP[s])

            for t in range(NTILE):
                s = 0 if t < NT_E else 1
                xq_t = xin.tile([P, D], I8, tag="xq")
                nc.sync.dma_start(xq_t[:], xq[t * P:(t + 1) * P, :])
                xbf = xbp.tile([P, D], BF16, tag="xbf")
                nc.vector.tensor_copy(xbf[:], xq_t[:])
                gx = gxp.tile([P, KC, P], BF16, tag="gx")
                for c in range(KC):
                    tp = pt.tile([P, P], BF16, tag="tp")
                    nc.tensor.transpose(tp[:], xbf[:, c * P:(c + 1) * P],
                                        ident[:])
                    nc.vector.tensor_copy(gx[:, c, :], tp[:])
                ps0 = pm.tile([P, 512], FP32, tag="ps0")
                ps1 = pm.tile([P, 512], FP32, tag="ps1")
                for c in range(KC):
                    nc.tensor.matmul(ps0[:], gx[:, c, :],
                                     w_sb[:, s, c, 0:512],
                                     start=(c == 0), stop=(c == KC - 1))
                    nc.tensor.matmul(ps1[:], gx[:, c, :],
                                     w_sb[:, s, c, 512:D],
                                     start=(c == 0), stop=(c == KC - 1))
                # y = psum * s_tok (ACT engine) + be (DVE, in-place fp32)
                of32 = ofp.tile([P, D], FP32, tag="of32")
                nc.scalar.activation(of32[:, 0:512], ps0[:],
                                     mybir.ActivationFunctionType.Copy,
                                     scale=sx_sb[:, t:t + 1])
                nc.scalar.activation(of32[:, 512:D], ps1[:],
                                     mybir.ActivationFunctionType.Copy,
                                     scale=sx_sb[:, t:t + 1])
                nc.vector.tensor_add(of32[:, 0:512], of32[:, 0:512],
                                     be_sb[:, s, 0:512])
                nc.vector.tensor_add(of32[:, 512:D], of32[:, 512:D],
                                     be_sb[:, s, 512:D])
                # per-token abs-max -> scale; requantize to uint8 (+128 offset)
                ya = yap.tile([P, D], FP32, tag="ya")
                nc.scalar.activation(ya[:], of32[:],
                                     mybir.ActivationFunctionType.Abs)
                mx8 = scp.tile([P, 8], FP32, tag="mx8")
                nc.vector.max(mx8[:], ya[:])
                nc.vector.tensor_scalar(so_all[:, t:t + 1], mx8[:, 0:1],
                                        1.0 / QMAX, None,
                                        op0=mybir.AluOpType.mult)
                inv = scp.tile([P, 1], FP32, tag="inv")
                nc.vector.reciprocal(inv[:], so_all[:, t:t + 1])
                o = op.tile([P, D], U8, tag="o")
                nc.scalar.activation(o[:, 0:512], of32[:, 0:512],
                                     mybir.ActivationFunctionType.Copy,
                                     scale=inv[:], bias=QBIAS)
                nc.scalar.activation(o[:, 512:D], of32[:, 512:D],
                                     mybir.ActivationFunctionType.Copy,
                                     scale=inv[:], bias=QBIAS)
                nc.sync.dma_start(out[t * P:(t + 1) * P, :], o[:])
            nc.sync.dma_start(soT[:], so_all[:])

    nc.compile()
    return nc


def _get_state():
    if _STATE.get("ready"):
        return _STATE
    _b2j.install_neuronx_cc_hook()
    nc = build_nc()
    devs = jax.devices()[:N_CORES]
    mesh = Mesh(np.asarray(devs), ("c",))
    shard = NamedSharding(mesh, PartitionSpec("c"))

    partition_name = (nc.partition_id_tensor.name
                      if nc.partition_id_tensor is not None else None)
    in_names, out_names, out_avals = [], [], []
    for alloc in nc.m.functions[0].allocations:
        if not isinstance(alloc, mybir.MemoryLocationSet):
            continue
        name = alloc.memorylocations[0].name
        if alloc.kind == "ExternalInput":
            if name != partition_name:
                in_names.append(name)
        elif alloc.kind == "ExternalOutput":
            out_names.append(name)
            out_avals.append(jax.core.ShapedArray(
                tuple(alloc.tensor_shape), mybir.dt.np(alloc.dtype)))
    n_params = len(in_names)
    all_names = in_names + out_names
    if partition_name is not None:
        all_names = all_names + [partition_name]
    donate = tuple(range(n_params, n_params + len(out_names)))

    def _body(*args):
        operands = list(args)
        if partition_name is not None:
            operands.append(_b2j.partition_id_tensor())
        outs = _b2j._bass_exec_p.bind(
            *operands,
            out_avals=tuple(out_avals),
            in_names=tuple(all_names),
            out_names=tuple(out_names),
            lowering_input_output_aliases=(),
            sim_require_finite=True,
            sim_require_nnan=True,
            nc=nc,
        )
        return tuple(outs)

    from jax.experimental.shard_map import shard_map
    sharded = jax.jit(
        shard_map(_body, mesh=mesh,
                  in_specs=(PartitionSpec("c"),) * (n_params + len(out_names)),
                  out_specs=(PartitionSpec("c"),) * len(out_names),
                  check_rep=False),
        donate_argnums=donate, keep_unused=True)

    # per-device variant for pipelined upload/exec/download
    single = jax.jit(_body, donate_argnums=donate, keep_unused=True)

    zeros_jit = jax.jit(
        lambda: tuple(jnp.zeros((N_CORES * a.shape[0],) + a.shape[1:], a.dtype)
                      for a in out_avals),
        out_shardings=tuple(shard for _ in out_avals))

    _STATE.update(ready=True, nc=nc, devs=devs, mesh=mesh, shard=shard,
                  in_names=in_names, out_names=out_names, out_avals=out_avals,
                  sharded=sharded, single=single, zeros_jit=zeros_jit,
                  wcache=None,
                  buf=np.empty((N_TOK, D), np.float32),
                  xq8=np.empty((N_TOK, D), np.int8),
                  yu8=np.empty((N_TOK, D), np.uint8),
                  ydiff=np.empty((N_TOK, D), np.int16),
                  so_full=np.empty(N_TOK, np.float32),
                  y=np.empty((N_TOK, D), np.float32))
    return _STATE


def _global_from_shards(st, shards, shape, dtype):
    """Assemble a sharded global jax array from 8 per-device host arrays."""
    arrs = [jax.device_put(shards[c], st["devs"][c]) for c in range(N_CORES)]
    gshape = (N_CORES * shape[0],) + tuple(shape[1:])
    return jax.make_array_from_single_device_arrays(gshape, st["shard"], arrs)


def _prep_weights(st, Wg, bg, We, be):
    """Device-resident wePT/beP, cached across calls on array equality."""
    wc = st.get("wcache")
    if wc is not None and np.array_equal(wc["We"], We) and \
            np.array_equal(wc["be"], be):
        return wc["wePT_g"], wc["beP_g"]
    # wePT[e][p][c*D+d] = We[e][d, c*128+p]
    weT = We.transpose(0, 2, 1)                            # [E, k, d]
    wePT = np.ascontiguousarray(
        weT.reshape(E, KC, P, D).transpose(0, 2, 1, 3).reshape(E, P, KC * D)
    ).astype(ml_dtypes.bfloat16)
    beP = np.ascontiguousarray(
        np.broadcast_to(be[:, None, :], (E, P, D))).astype(np.float32)
    wePT_g = _global_from_shards(
        st, [wePT[c * EPC:(c + 1) * EPC] for c in range(N_CORES)],
        (EPC, P, KC * D), ml_dtypes.bfloat16)
    beP_g = _global_from_shards(
        st, [beP[c * EPC:(c + 1) * EPC] for c in range(N_CORES)],
        (EPC, P, D), np.float32)
    st["wcache"] = dict(We=We.copy(), be=be.copy(), wePT_g=wePT_g, beP_g=beP_g)
    return wePT_g, beP_g


def kernel(x, Wg, bg, We, be):
    tt = [("start", time.time())]

    def _tick(name):
        tt.append((name, time.time()))

    x = np.asarray(x, dtype=np.float32)
    Wg = np.asarray(Wg, dtype=np.float32)
    bg = np.asarray(bg, dtype=np.float32)
    We = np.asarray(We, dtype=np.float32)
    be = np.asarray(be, dtype=np.float32)
    assert x.shape == (N_TOK, D) and We.shape == (E, D, D), (x.shape, We.shape)

    st = _get_state()
    _tick("state")
    wePT_g, beP_g = _prep_weights(st, Wg, bg, We, be)
    _tick("weights")

    # ---- fp32 routing on host (matches reference bit-for-bit in practice)
    logits = x @ Wg.T
    logits += bg
    idx = np.argmax(logits, axis=1).astype(np.int32)
    _tick("routing")

    # ---- per-token int8 quantization (preallocated buffers)
    buf, xq8 = st["buf"], st["xq8"]
    s = np.abs(x).max(axis=1)
    s /= 127.0
    np.maximum(s, 1e-30, out=s)
    np.multiply(x, (1.0 / s)[:, None], out=buf)
    np.rint(buf, out=buf)
    np.copyto(xq8, buf, casting="unsafe")
    _tick("quant")

    # ---- dispatch: slot tables per core (expert e -> core e//2, slot e%2)
    order = np.argsort(idx, kind="stable")
    counts = np.bincount(idx, minlength=E)
    starts = np.zeros(E + 1, np.int64)
    np.cumsum(counts, out=starts[1:])
    tok_by_e = [order[starts[e]:starts[e + 1]] for e in range(E)]
    overflow = []                                  # (expert, token-array)
    for e in range(E):
        if counts[e] > CAP_E:
            overflow.append((e, tok_by_e[e][CAP_E:]))
            tok_by_e[e] = tok_by_e[e][:CAP_E]

    # ---- pipelined per-core: build -> upload -> exec -> download, overlapped
    devs = st["devs"]
    zeros = st["zeros_jit"]()
    zparts = [sorted(z.addressable_shards, key=lambda sd: sd.index[0].start)
              for z in zeros]
    wparts = sorted(wePT_g.addressable_shards, key=lambda sd: sd.index[0].start)
    bparts = sorted(beP_g.addressable_shards, key=lambda sd: sd.index[0].start)
    name_pos = {n: i for i, n in enumerate(st["in_names"])}
    single = st["single"]
    out_pos = {n: i for i, n in enumerate(st["out_names"])}

    y = st["y"]
    yu8 = st["yu8"]
    so_full = st["so_full"]
    fetch_t = np.zeros(N_CORES)
    scat_t = np.zeros(N_CORES)
    outs_pc = [None] * N_CORES
    threads = []

    def _fetch(c):
        t0 = time.time()
        part = np.asarray(outs_pc[c][out_pos["out"]])   # [CAP_C, D] uint8
        soT = np.asarray(outs_pc[c][out_pos["soT"]])    # [P, NTILE] fp32
        t1 = time.time()
        so = soT.T.reshape(CAP_C)
        for sl in range(EPC):
            tk = tok_by_e[c * EPC + sl]
            yu8[tk] = part[sl * CAP_E:sl * CAP_E + len(tk)]
            so_full[tk] = so[sl * CAP_E:sl * CAP_E + len(tk)]
        fetch_t[c] = t1 - t0
        scat_t[c] = time.time() - t1

    for c in range(N_CORES):
        xq_pad = np.zeros((CAP_C, D), np.int8)
        s_pad = np.zeros(CAP_C, np.float32)
        for sl in range(EPC):
            tk = tok_by_e[c * EPC + sl]
            xq_pad[sl * CAP_E:sl * CAP_E + len(tk)] = xq8[tk]
            s_pad[sl * CAP_E:sl * CAP_E + len(tk)] = s[tk]
        sxT = np.ascontiguousarray(s_pad.reshape(NTILE, P).T)
        args = [None] * len(st["in_names"])
        args[name_pos["xq"]] = jax.device_put(xq_pad, devs[c])
        args[name_pos["sxT"]] = jax.device_put(sxT, devs[c])
        args[name_pos["wePT"]] = wparts[c].data
        args[name_pos["beP"]] = bparts[c].data
        outs_pc[c] = single(*args, *[zp[c].data for zp in zparts])
        th = threading.Thread(target=_fetch, args=(c,))
        th.start()
        threads.append(th)
        if c == 0:
            _tick("dispatch_build")
    for t in threads:
        t.join()
    _tick("exec_download")

    # single fused dequant pass: y = (u8 - 128) * so
    np.subtract(yu8, np.uint8(128), out=st["ydiff"], dtype=np.int16,
                casting="unsafe")
    np.multiply(st["ydiff"], so_full[:, None], out=y)

    # ---- host fallback for capacity overflow (a few dozen rows)
    for e, tk in overflow:
        y[tk] = x[tk] @ We[e].T + be[e]

    _tick("download_scatter")
    kernel.last_results = None
    if os.environ.get("MOE_TIME"):
        for (n0, t0), (n1, t1) in zip(tt, tt[1:]):
            print(f"  [{n1}] {t1 - t0:.3f}s")
        print(f"  [total] {tt[-1][1] - tt[0][1]:.3f}s")
        print(f"  fetch={fetch_t.sum():.3f}s(sum) scat={scat_t.sum():.3f}s(sum)")
    return y
